# revision 27
# baseline (speedup 1.0000x reference)
"""Trainium2 Bass kernel for NT-Xent contrastive loss (N=4096, D=256).

loss = mean_i(log(sum_{k!=i} exp(sim(r_i,r_k)/T)) - sim(r_i, r_{i+N mod 2N})/T)
with r = row-l2-normalized concat(emb_i, emb_j), T = 0.5.

Symmetric block-triangle sharding across 8 cores: the 8192 rows form 8
blocks of 1024. Core c owns row-block c. Of the 8192x8192 exp(Gram)
matrix, each unordered block pair is computed once (its transpose
direction is recovered from column sums), so each core evaluates only
36 [128x1024] strip-units instead of 64:

  - diag block (c,c):        8 strips, row sums only (self term is
    subtracted analytically on the host from the fp8-exact norms)
  - blocks (c,c+1..c+3):     24 strips; row sums via the ACT
    accumulator, column sums -- which are the (c+k,c) blocks' row
    contributions by symmetry -- via DVE adds of the bf16 exp tiles
  - gap block pair {c,c+4}:  split by row m-component: core c takes
    rows with m in 0..3 (full 1024 cols), core c+4 takes all its rows
    x cols with t in 4..7 (512 cols). Both shapes are emitted as 8
    uniform [128x512] strips whose lhsT/rhs come from per-core
    host-packed gap regions, keeping the program SPMD-uniform.

Matmuls run in fp8e4 DoubleRow mode (K=256 packed as 2 k-subtiles of
128 -> 2x PE throughput). The Scalar engine does nothing but exp (one
activation-table load): 24 exp instructions with row-sum accumulators,
~40us busy, which is the roofline for this decomposition. The host
performs only O(N*D) input prep (normalize, transpose, fp8 cast,
positive-pair dots) and O(N) finalization (partial sums, final log);
all O(N^2) work is on device.
"""

import os
import numpy as np
import ml_dtypes

import concourse.bass as bass
import concourse.bacc as bacc
import concourse.tile as tile
from concourse import mybir
from concourse.bass_utils import run_bass_kernel_spmd
from contextlib import ExitStack

N = 4096
D = 256
TWO_N = 2 * N
N_CORES = 8
NB = 8                 # row/col blocks
BLK = TWO_N // NB      # 1024 rows per block
TPB = BLK // 128       # 8 tiles per block (row-in-block = 8p + m)
NJ = 5                 # column blocks held per core: c, c+1, .., c+4
KC = 2                 # K=256 = 2 k-subtiles of 128

F32 = mybir.dt.float32
BF16 = mybir.dt.bfloat16
FP8 = mybir.dt.float8e4
ALU = mybir.AluOpType
ACT = mybir.ActivationFunctionType
AXX = mybir.AxisListType
DR = mybir.MatmulPerfMode.DoubleRow


def _emit(nc, tc, ctx, repsT_in, gapL_in, gapR_in, out1, outA, outA4, outAd):
    persist = ctx.enter_context(tc.tile_pool(name="persist", bufs=1))
    work = ctx.enter_context(tc.tile_pool(name="work", bufs=3))
    psum = ctx.enter_context(tc.tile_pool(name="psum", bufs=2, space="PSUM"))

    # one SBUF tile per column block: contiguous per partition, so each
    # load is a single descriptor per partition (fast SWDGE generation)
    rT = [persist.tile([128, KC, TPB, 128], FP8, name=f"rT{j}")
          for j in range(NJ)]
    gapL = persist.tile([128, KC, TPB, 128], FP8)
    gapR = persist.tile([128, KC, 4 * TPB, 128], FP8)
    # den accum: 0..7 S0 {diag-tri,c+1}, 8..15 S1 {c+2,c+3}, 16..23 gap
    # strips, 24 = second half of the split m=0 S0 strip
    scalars = persist.tile([128, 25], F32)
    A = persist.tile([128, 3, BLK], BF16)
    A4 = persist.tile([128, 2, 512], BF16)
    Ad = persist.tile([128, 7, 128], BF16)

    # ---- loads (own block first so the first strip starts ASAP) ----
    for j in range(NJ):
        nc.sync.dma_start(out=rT[j][:, :, :, :], in_=repsT_in.ap()[:, j])
    nc.sync.dma_start(out=gapL[:, :, :, :], in_=gapL_in.ap())
    nc.sync.dma_start(out=gapR[:, :, :, :], in_=gapR_in.ap())

    def strip(m, specs, den_slot):
        """specs: list of (block_tile, local_t0, ntile). Returns exp tile."""
        width = sum(s[2] for s in specs) * 128
        ps = psum.tile([128, 2048], F32, tag="mm")
        col = 0
        for buf, t0, nt in specs:
            for i in range(0, nt, 2):
                w = min(2, nt - i)
                nc.tensor.matmul(out=ps[:, col:col + w * 128],
                                 lhsT=rT[0][:, :, m, :],
                                 rhs=buf[:, :, t0 + i:t0 + i + w, :],
                                 start=True, stop=True, perf_mode=DR)
                col += w * 128
        e = work.tile([128, 2048], BF16, tag="E")
        nc.scalar.activation(out=e[:, :width], in_=ps[:, :width],
                             func=ACT.Exp, scale=2.0,
                             accum_out=scalars[:, den_slot:den_slot + 1])
        return e

    def acc_A(dst_ap, src_ap, first):
        if first:
            nc.vector.tensor_copy(dst_ap, src_ap)
        else:
            nc.vector.tensor_tensor(out=dst_ap, in0=dst_ap, in1=src_ap,
                                    op=ALU.add)

    # ---- S0: cols = {diag tiles m..7, c+1}; the diag block is its own
    # transpose, so only its tile-level upper triangle is computed; col
    # sums of tiles t>m -> Ad, col sums of c+1 -> A[0]. m=0 is split in
    # two strips so the first exp only waits on the first DMA chunk. ----
    e = strip(0, [(rT[0], 0, TPB)], 0)
    acc_A(Ad[:, :, :].rearrange("p a b -> p (a b)"), e[:, 128:1024], True)
    e = strip(0, [(rT[1], 0, TPB)], 24)
    acc_A(A[:, 0, :], e[:, :BLK], True)
    # ---- S1: cols = {c+2, c+3} -> A[1], A[2]. Emitted before the
    # ragged S0 strips so strip widths are descending: a strip's matmuls
    # then always fit under the previous strip's exp (no PE bubbles). ----
    for m in range(TPB):
        e = strip(m, [(rT[2], 0, TPB), (rT[3], 0, TPB)], 8 + m)
        acc_A(A[:, 1:3, :].rearrange("p a b -> p (a b)"), e[:, :], m == 0)

    # ragged diag section last so every matmul output stays 256-aligned
    # within a PSUM bank
    for m in range(1, TPB):
        nd = TPB - m
        e = strip(m, [(rT[1], 0, TPB), (rT[0], m, nd)], m)
        acc_A(A[:, 0, :], e[:, 0:BLK], False)
        if m < 7:
            acc_A(Ad[:, m:, :].rearrange("p a b -> p (a b)"),
                  e[:, BLK + 128:BLK + nd * 128], False)

    # ---- gap strips: 8 uniform [128, 512], host-packed lhsT/rhs ----
    for k in range(TPB):
        ps = psum.tile([128, 2048], F32, tag="mm")
        for i in range(0, 4, 2):
            nc.tensor.matmul(out=ps[:, i * 128:(i + 2) * 128],
                             lhsT=gapL[:, :, k, :],
                             rhs=gapR[:, :, 4 * k + i:4 * k + i + 2, :],
                             start=True, stop=True, perf_mode=DR)
        e = work.tile([128, 2048], BF16, tag="E")
        nc.scalar.activation(out=e[:, :512], in_=ps[:, :512],
                             func=ACT.Exp, scale=2.0)
        # gap row sums on DVE: skips the 183ns ACT accumulator read
        nc.vector.tensor_reduce(out=scalars[:, 16 + k:17 + k],
                                in_=e[:, :512], axis=AXX.X, op=ALU.add)
        acc_A(A4[:, k // 4, :], e[:, :512], k % 4 == 0)
        if k == 4:
            # A[0:3] and Ad final after S1's last add: ship early
            nc.sync.dma_start(out=outA.ap(), in_=A[:, :, :])
        elif k == 5:
            nc.sync.dma_start(out=outAd.ap(), in_=Ad[:, :, :])
            nc.sync.dma_start(out=outA4.ap()[:, 0:1, :], in_=A4[:, 0:1, :])
            # S0/S1 den slots are final too
            nc.sync.dma_start(out=out1.ap()[:, 0:16], in_=scalars[:, 0:16])

    nc.sync.dma_start(out=outA4.ap()[:, 1:2, :], in_=A4[:, 1:2, :])
    nc.sync.dma_start(out=out1.ap()[:, 16:25], in_=scalars[:, 16:25])


_CACHED = None


def _build():
    global _CACHED
    if _CACHED is not None:
        return _CACHED
    nc = bacc.Bacc("TRN2", target_bir_lowering=False, debug=False,
                   enable_asserts=False, num_devices=N_CORES)
    repsT_in = nc.dram_tensor("repsT_in", [128, NJ, KC, TPB, 128], FP8,
                              kind="ExternalInput")
    gapL_in = nc.dram_tensor("gapL_in", [128, KC, TPB, 128], FP8,
                             kind="ExternalInput")
    gapR_in = nc.dram_tensor("gapR_in", [128, KC, 4 * TPB, 128], FP8,
                             kind="ExternalInput")
    out1 = nc.dram_tensor("out1", [128, 25], F32, kind="ExternalOutput")
    outA = nc.dram_tensor("outA", [128, 3, BLK], BF16, kind="ExternalOutput")
    outA4 = nc.dram_tensor("outA4", [128, 2, 512], BF16,
                           kind="ExternalOutput")
    outAd = nc.dram_tensor("outAd", [128, 7, 128], BF16,
                           kind="ExternalOutput")
    with tile.TileContext(nc) as tc:
        with ExitStack() as ctx:
            _emit(nc, tc, ctx, repsT_in, gapL_in, gapR_in, out1, outA, outA4,
                  outAd)
    nc.compile()
    _CACHED = nc
    return nc


def _prep(emb_i, emb_j):
    """Host O(N*D) prep: normalize, fp8-quantize, transpose into the
    DoubleRow k-tile layout, pack per-core gap regions, pos dots."""
    reps = np.concatenate([np.asarray(emb_i, dtype=np.float64),
                           np.asarray(emb_j, dtype=np.float64)], axis=0)
    rho = reps / np.maximum(np.linalg.norm(reps, axis=1, keepdims=True),
                            1e-12)
    pos_logits = 2.0 * np.sum(rho * np.roll(rho, N, axis=0), axis=1)

    rho8 = rho.astype(np.float32).astype(ml_dtypes.float8_e4m3)
    # self-sim exactly as the fp8 matmul computes it
    r8f = rho8.astype(np.float64)
    self_sim = np.sum(r8f * r8f, axis=1)

    # repsT[p, kc, J*TPB+m, q] = rho8[1024J + 8q + m, kc*128 + p]
    # R2[J, q, m, kc, p] -> transpose to [J, p, kc, m, q]
    R2 = rho8.reshape(NB, 128, TPB, KC, 128).transpose(0, 4, 3, 2, 1)
    R2 = np.ascontiguousarray(R2)       # [NB, 128, KC, TPB, 128]

    in_maps = []
    for c in range(N_CORES):
        js = [(c + k) % NB for k in range(NJ)]
        repsT = np.ascontiguousarray(
            np.stack([R2[j] for j in js], axis=1))   # [128, NJ, KC, TPB, 128]
        partner = R2[(c + 4) % NB]                   # [128, KC, TPB, 128]
        own = R2[c]
        if c < 4:
            # rows m = k mod 4, cols = partner tiles 0..3 (k<4) / 4..7
            gapL = own[:, :, [0, 1, 2, 3, 0, 1, 2, 3], :]
            gapR = np.stack(
                [partner[:, :, (0 if k < 4 else 4) + i, :]
                 for k in range(TPB) for i in range(4)], axis=2)
        else:
            # rows m = k, cols = partner tiles 4..7 always
            gapL = own
            gapR = np.stack(
                [partner[:, :, 4 + i, :]
                 for _ in range(TPB) for i in range(4)], axis=2)
        in_maps.append({
            "repsT_in": repsT,
            "gapL_in": np.ascontiguousarray(gapL),
            "gapR_in": np.ascontiguousarray(gapR),
        })
    return in_maps, pos_logits, self_sim


def _combine(results, pos_logits, self_sim):
    den = np.zeros(TWO_N, dtype=np.float64)
    for c in range(N_CORES):
        o1 = np.asarray(results[c]["out1"], dtype=np.float64)      # [128, 24]
        Ac = np.asarray(results[c]["outA"].astype(np.float32),
                        dtype=np.float64)                          # [128,3,1024]
        rows = slice(BLK * c, BLK * (c + 1))
        # S0 + S1 strips: rows 8p+m (slot 24 = second half of S0 m=0)
        s0 = o1[:, 0:8].copy()
        s0[:, 0] += o1[:, 24]
        den[rows] += (s0 + o1[:, 8:16]).reshape(BLK)
        # diag-triangle col sums: Ad[:, t-1, q] -> row 8q+t of block c
        Adc = np.asarray(results[c]["outAd"].astype(np.float32),
                         dtype=np.float64)                         # [128,7,128]
        dd = den[rows].reshape(128, TPB)
        dd[:, 1:8] += Adc.sum(axis=0).T
        # gap strips: row sums
        g = o1[:, 16:24]                                           # [128, k]
        add = np.zeros((128, TPB))
        if c < 4:
            for k in range(TPB):
                add[:, k % 4] += g[:, k]
        else:
            add = g
        den[rows] += add.reshape(BLK)
        # gap strips: column sums -> partner block rows
        A4 = np.asarray(results[c]["outA4"].astype(np.float32),
                        dtype=np.float64)                          # [128,2,512]
        Jg = (c + 4) % NB
        dg = den[BLK * Jg:BLK * (Jg + 1)].reshape(128, TPB)        # [q, 8q+t]
        if c < 4:
            # halves cover partner tiles 0..3 and 4..7
            cp = A4.sum(axis=0).reshape(2, 4, 128)                 # [h, tl, q]
            dg[:, 0:4] += cp[0].T
            dg[:, 4:8] += cp[1].T
        else:
            # both halves cover partner tiles 4..7
            cp = (A4[:, 0, :] + A4[:, 1, :]).sum(axis=0).reshape(4, 128)
            dg[:, 4:8] += cp.T
        # column-sum partials: A[k-1] -> block c+k rows
        for k in (1, 2, 3):
            J = (c + k) % NB
            cp = Ac[:, k - 1, :].sum(axis=0)        # [1024] indexed (t*128+q)
            den[BLK * J:BLK * (J + 1)] += cp.reshape(TPB, 128).T.reshape(BLK)
    # subtract the diagonal self term
    den -= np.exp(2.0 * self_sim)
    return float(np.mean(np.log(den) - pos_logits))


LAST_EXEC_NS = None
LAST_TRACE = None


def kernel(emb_i, emb_j, batch_size):
    global LAST_EXEC_NS, LAST_TRACE
    emb_i = np.ascontiguousarray(np.asarray(emb_i), dtype=np.float32)
    emb_j = np.ascontiguousarray(np.asarray(emb_j), dtype=np.float32)
    assert emb_i.shape == (N, D) and emb_j.shape == (N, D)

    nc = _build()
    in_maps, pos_logits, self_sim = _prep(emb_i, emb_j)
    trace = bool(int(os.environ.get("KERNEL_TRACE", "0")))
    res = run_bass_kernel_spmd(nc, in_maps, list(range(N_CORES)), trace=trace)
    LAST_EXEC_NS = res.exec_time_ns
    if res.instructions_and_trace is not None:
        LAST_TRACE = res.instructions_and_trace[1]

    return np.array(_combine(res.results, pos_logits, self_sim),
                    dtype=np.float32)
